# revision 15
# baseline (speedup 1.0000x reference)
"""Trainium2 Bass kernel for nn_CausalLSTMNodeCell (B=1048576, D=32, H=16, C=3).

Strategy: pure data parallel over the batch across 8 cores, with output rows
organized by residue b mod 3 so the TF-row-major child_r reshape becomes three
extra contiguous-row matmuls. Per 128-row block (K=98, block-diagonal rhs):
  psum[:, 0:32]   = xtB.T @ WB[rho]   (r1, r2 gates)
  psum[:, 32:144] = xtA.T @ WA[rho]   (r0, a, ifo, n1, n2)
All gates (incl. tanh'd "a", via tanh z = 2*sigmoid(2z)-1 with host-prescaled
weights) come out of one Sigmoid per psum group on ScalarE. Each block's psum
slice is 256 f32 wide so every matmul output stays inside one 2KB PSUM bank.

v3: fp8 E3M4 for BOTH matmul operands (uniform dtype; mixed fp8xbf16 is a
slow/incorrect path on this stack). Weights are pre-scaled x32 so they sit in
e3m4's normal range (sigmoid applies scale=1/32 for free); the precision-
critical A/I/F/O columns get a second fp8 "lo" residual matmul accumulating
into the same psum (error ~0.2%, emulated rel err n .0095 / h .012 / c .007).
Elementwise is bf16 with a reordered gate layout [r1 r2 | r0 a i f o n1 n2]:
the three r-gate muls fuse into ONE 48-col GPSIMD op, [n1 n2]*[r h] into one
32-col DVE op. Supergroups of 32 blocks are software-pipelined with LAG=1;
loads ride the SP HWDGE ring, stores the ACT ring.
"""

import numpy as np
import ml_dtypes

B, D, H, C = 1048576, 32, 16, 3
NCORES = 8
R = B // NCORES            # 131072 rows per core
TPB = 342                  # blocks per residue section (padded, 43776>=43691)
TP = TPB * 128             # rows per section
NBLK = 3 * TPB             # blocks per core
SG = 32                    # max blocks per supergroup
GRP = 8                    # blocks per psum group
SG_TABLE = []              # (blk0, size) pairs
for _rho in range(3):
    _off = _rho * TPB
    for _i in range(TPB // SG):
        SG_TABLE.append((_off + _i * SG, SG))
    if TPB % SG:
        SG_TABLE.append((_off + (TPB // SG) * SG, TPB % SG))
NSG = len(SG_TABLE)
KDIM = 98
bf16 = ml_dtypes.bfloat16
f8e3 = ml_dtypes.float8_e3m4

XH_FP8 = True              # ship xta/xtb as fp8 e3m4
CN_FP8 = False             # ship child_n as fp8 e3m4
W_FP8 = True               # weights as fp8 e3m4 at x32 scale (sigmoid 1/32)
W_HILO = True              # fp8 hi+lo residual matmul for the A/I/F/O columns
W_SCALE = 32.0
EL16 = True                # bf16 elementwise tiles (False: f32 like v1)
XT_BUFS = 3                # buffer depth for xtA/xtB load tiles
AXD_BUFS = 4               # buffer depth for c_prev/child load tiles
LAG = 1                    # software-pipeline depth (supergroups)

_NC_CACHE = {}


def _build_w(inputs):
    W = np.zeros((49, 144), np.float32)

    def put(cols, wx, wh, bx, bh):
        W[0:32, cols] = inputs[wx]
        W[32:48, cols] = inputs[wh]
        W[48, cols] = inputs[bx] + inputs[bh]

    put(slice(0, 48), "W_ifo_x", "W_ifo_h", "b_ifo_x", "b_ifo_h")
    put(slice(48, 64), "W_n1_x", "W_n1_h", "b_n1_x", "b_n1_h")
    put(slice(64, 80), "W_n2_x", "W_n2_h", "b_n2_x", "b_n2_h")
    put(slice(80, 96), "W_a_x", "W_a_h", "b_a_x", "b_a_h")
    put(slice(96, 144), "W_r_x", "W_r_h", "b_r_x", "b_r_h")
    return W


def host_prep(inputs):
    x = np.asarray(inputs["inputs"], np.float32)
    hp = np.asarray(inputs["h_prev"], np.float32)
    cp = np.asarray(inputs["c_prev"], np.float32)
    ch = np.asarray(inputs["child_n"], np.float32)
    W = _build_w(inputs)
    Wr = W[:, 96:144]
    xh = np.zeros((B + 1, 49), np.float32)
    xh[:B, 0:32] = x
    xh[:B, 32:48] = hp
    xh[:B, 48] = 1.0
    xhdt = f8e3 if XH_FP8 else bf16
    xh16 = xh.astype(xhdt)

    cndt = f8e3 if CN_FP8 else bf16
    cp16 = cp.astype(bf16)

    cores = []
    for m in range(NCORES):
        xtA = np.zeros((KDIM, 3 * TP), xhdt)
        xtB = np.zeros((KDIM, 3 * TP), xhdt)
        cpp = np.zeros((3 * TP, 16), bf16)
        # cn packed in gate order [r1 r2 r0] -> child columns [ch1 ch2 ch0]
        chp = np.zeros((3 * TP, 48), cndt)
        WA = np.zeros((3, KDIM, 112), np.float32)
        WB = np.zeros((3, KDIM, 32), np.float32)
        for rho in range(3):
            first = m * R + ((rho - m * R) % 3)
            T = len(range(first, (m + 1) * R, 3))
            sl = slice(rho * TP, rho * TP + TP)
            bidx = first + 3 * np.arange(TP)
            bidx = np.minimum(bidx, B)
            bidx[T:] = B
            xtA[0:49, sl] = xh16[bidx].T
            cpp[sl.start:sl.start + T] = cp16[first:(m + 1) * R:3]
            chp[sl.start:sl.start + T, 0:16] = \
                ch[1, first:(m + 1) * R:3].astype(cndt)
            chp[sl.start:sl.start + T, 16:32] = \
                ch[2, first:(m + 1) * R:3].astype(cndt)
            chp[sl.start:sl.start + T, 32:48] = \
                ch[0, first:(m + 1) * R:3].astype(cndt)
            q = [(k * 16 * B + 16 * first) // 48 for k in range(3)]
            c = [16 * ((k + rho) % 3) for k in range(3)]
            for k, dst, rows in ((0, xtA, slice(49, 98)),
                                 (1, xtB, slice(0, 49)),
                                 (2, xtB, slice(49, 98))):
                qi = np.minimum(q[k] + np.arange(TP), B)
                dst[rows, sl] = xh16[qi].T
            # gate column order (psum): [R1 R2 | R0 A I F O N1 N2]
            WA[rho, 49:98, 0:16] = Wr[:, c[0]:c[0] + 16]    # r0 -> psum 32:48
            # a-preact scaled by 2: tanh(z) = 2*sigmoid(2z) - 1
            WA[rho, 0:49, 16:32] = 2.0 * W[:, 80:96]        # a   -> 48:64
            WA[rho, 0:49, 32:80] = W[:, 0:48]               # ifo -> 64:112
            WA[rho, 0:49, 80:96] = W[:, 48:64]              # n1  -> 112:128
            WA[rho, 0:49, 96:112] = W[:, 64:80]             # n2  -> 128:144
            WB[rho, 0:49, 0:16] = Wr[:, c[1]:c[1] + 16]     # r1 -> 0:16
            WB[rho, 49:98, 16:32] = Wr[:, c[2]:c[2] + 16]   # r2 -> 16:32
        cpp = np.ascontiguousarray(
            cpp.reshape(NBLK, 128, 16).transpose(1, 0, 2))
        chp = np.ascontiguousarray(
            chp.reshape(NBLK, 128, 48).transpose(1, 0, 2))
        core = dict(xta=xtA, xtb=xtB, cpt=cpp, cnt=chp)
        if W_FP8:
            wa_hi = np.clip(W_SCALE * WA, -15.5, 15.5).astype(f8e3)
            wb_hi = np.clip(W_SCALE * WB, -15.5, 15.5).astype(f8e3)
            core["wa"] = wa_hi
            core["wb"] = wb_hi
            if W_HILO:
                # lo residual for WA cols 16:80 (A, I, F, O)
                lo = (W_SCALE * WA[:, :, 16:80]
                      - wa_hi[:, :, 16:80].astype(np.float32))
                core["wal"] = lo.astype(f8e3)
        else:
            core["wa"] = WA.astype(bf16)
            core["wb"] = WB.astype(bf16)
        cores.append(core)
    return cores


def build_nc(niter=1, sg_bufs=3):
    import concourse.tile as tile
    from concourse import bacc, mybir

    f32 = mybir.dt.float32
    b16 = mybir.dt.bfloat16
    eldt = b16 if EL16 else f32
    xhdt = mybir.dt.float8e3 if XH_FP8 else b16
    cndt = mybir.dt.float8e3 if CN_FP8 else b16
    wdt = mybir.dt.float8e3 if W_FP8 else b16
    AF = mybir.ActivationFunctionType
    ALU = mybir.AluOpType

    nc = bacc.Bacc(None, target_bir_lowering=False)
    xta_d = nc.dram_tensor("xta", [KDIM, 3 * TP], xhdt, kind="ExternalInput")
    xtb_d = nc.dram_tensor("xtb", [KDIM, 3 * TP], xhdt, kind="ExternalInput")
    wa_d = nc.dram_tensor("wa", [3, KDIM, 112], wdt, kind="ExternalInput")
    wb_d = nc.dram_tensor("wb", [3, KDIM, 32], wdt, kind="ExternalInput")
    if W_FP8 and W_HILO:
        wal_d = nc.dram_tensor("wal", [3, KDIM, 64], wdt, kind="ExternalInput")
    cpt_d = nc.dram_tensor("cpt", [128, NBLK, 16], b16, kind="ExternalInput")
    cnt_d = nc.dram_tensor("cnt", [128, NBLK, 48], cndt, kind="ExternalInput")
    # dram res columns: [n | c | h]
    res_d = nc.dram_tensor("res", [128, NBLK, 48], b16, kind="ExternalOutput")

    # gate columns (psum/GATES): [R1 R2 | R0 A I F O N1 N2]
    RRR = slice(0, 48)
    A = slice(48, 64)
    I, F, O = slice(64, 80), slice(80, 96), slice(96, 112)
    N12 = slice(112, 144)
    # tmp columns
    T0, T1, TC = slice(0, 16), slice(16, 32), slice(32, 48)
    P = slice(48, 96)
    P1, P2, P3 = slice(48, 64), slice(64, 80), slice(80, 96)
    # rh columns
    RS, HS = slice(0, 16), slice(16, 32)
    # res columns
    RN, RC = slice(0, 16), slice(16, 32)

    with tile.TileContext(nc) as tc:
        with (
            tc.tile_pool(name="wp", bufs=1) as wp,
            tc.tile_pool(name="xtab", bufs=XT_BUFS) as xtabp,
            tc.tile_pool(name="axd", bufs=AXD_BUFS) as axdp,
            tc.tile_pool(name="gates", bufs=sg_bufs) as gatesp,
            tc.tile_pool(name="tmp", bufs=sg_bufs) as tmpp,
            tc.tile_pool(name="res", bufs=sg_bufs) as resp,
            tc.tile_pool(name="ps", bufs=2, space="PSUM") as psp,
        ):
            wa_t = wp.tile([KDIM, 3, 112], wdt, tag="wa")
            wb_t = wp.tile([KDIM, 3, 32], wdt, tag="wb")
            wal_t = None
            if W_FP8 and W_HILO:
                wal_t = wp.tile([KDIM, 3, 64], wdt, tag="wal", name="wal_t")
            for rho in range(3):
                nc.sync.dma_start(wa_t[:, rho, :], wa_d[rho])
                nc.sync.dma_start(wb_t[:, rho, :], wb_d[rho])
                if wal_t is not None:
                    nc.sync.dma_start(wal_t[:, rho, :], wal_d[rho])

            V = nc.vector
            G = nc.gpsimd

            def gate_phase(s):
                blk0, sz = SG_TABLE[s]
                rho = blk0 // TPB
                col0 = blk0 * 128
                xta_t = xtabp.tile([KDIM, sz * 128], xhdt, tag="xta")
                nc.sync.dma_start(xta_t[:], xta_d[:, col0:col0 + sz * 128])
                xtb_t = xtabp.tile([KDIM, sz * 128], xhdt, tag="xtb")
                nc.sync.dma_start(xtb_t[:], xtb_d[:, col0:col0 + sz * 128])
                cpt = axdp.tile([128, sz, 16], b16, tag="cpt")
                nc.sync.dma_start(cpt[:], cpt_d[:, blk0:blk0 + sz, :])
                cnt = axdp.tile([128, sz, 48], cndt, tag="cnt")
                nc.sync.dma_start(cnt[:], cnt_d[:, blk0:blk0 + sz, :])

                gates = gatesp.tile([128, sz, 144], eldt, tag="gates")
                tmp = tmpp.tile([128, sz, 96], eldt, tag="tmp")
                rh = tmpp.tile([128, sz, 32], b16, tag="rh")
                nm = tmpp.tile([128, sz, 32], eldt, tag="nm")
                res = resp.tile([128, sz, 32], b16, tag="res")
                for g in range(-(-sz // GRP)):
                    gsz = min(GRP, sz - g * GRP)
                    # 256 f32 per block: each matmul output slice stays
                    # inside one 2KB PSUM bank (144-wide tiling would make
                    # blocks 3/6/10/13 span banks)
                    ps = psp.tile([128, gsz, 256], f32, tag="ps")
                    for bb in range(gsz):
                        k = g * GRP + bb
                        nc.tensor.matmul(
                            ps[:, bb, 0:32],
                            xtb_t[:, k * 128:(k + 1) * 128],
                            wb_t[:, rho, :])
                        nc.tensor.matmul(
                            ps[:, bb, 32:144],
                            xta_t[:, k * 128:(k + 1) * 128],
                            wa_t[:, rho, :],
                            start=True, stop=not (W_FP8 and W_HILO),
                            skip_group_check=True)
                        if W_FP8 and W_HILO:
                            # lo residual accumulates onto psum cols 48:112
                            # (A, I, F, O)
                            nc.tensor.matmul(
                                ps[:, bb, 48:112],
                                xta_t[:, k * 128:(k + 1) * 128],
                                wal_t[:, rho, :],
                                start=False, stop=True,
                                skip_group_check=True)
                    gsl = slice(g * GRP, g * GRP + gsz)
                    if W_FP8:
                        nc.scalar.activation(
                            gates[:, gsl, 0:144], ps[:, :, 0:144],
                            AF.Sigmoid, scale=1.0 / W_SCALE)
                    else:
                        nc.scalar.activation(
                            gates[:, gsl, 0:144], ps[:, :, 0:144], AF.Sigmoid)
                # c chain: a' = 2*sig_a - 1 ; t0 = a'*i ; t1 = f*cp ; c = t0+t1
                sl = slice(0, sz)
                V.tensor_scalar(tmp[:, sl, TC], gates[:, sl, A], 2.0,
                                -1.0, ALU.mult, ALU.add)
                V.tensor_mul(tmp[:, sl, T0], tmp[:, sl, TC], gates[:, sl, I])
                V.tensor_mul(tmp[:, sl, T1], gates[:, sl, F], cpt[:, sl, :])
                V.tensor_add(res[:, sl, RC], tmp[:, sl, T0], tmp[:, sl, T1])
                return (gates, cpt, cnt, tmp, rh, nm, res, blk0, sz)

            def elem_phase(state):
                gates, cpt, cnt, tmp, rh, nm, res, blk0, sz = state
                nc.scalar.activation(tmp[:, :, TC], res[:, :, RC], AF.Tanh)
                V.tensor_mul(rh[:, :, HS], gates[:, :, O], tmp[:, :, TC])
                G.tensor_mul(tmp[:, :, P], gates[:, :, RRR], cnt[:, :, :])
                V.tensor_add(tmp[:, :, T0], tmp[:, :, P1], tmp[:, :, P2])
                V.tensor_add(rh[:, :, RS], tmp[:, :, T0], tmp[:, :, P3])
                V.tensor_mul(nm[:, :, :], gates[:, :, N12], rh[:, :, :])
                V.tensor_add(res[:, :, RN], nm[:, :, 0:16], nm[:, :, 16:32])
                # stores ride the ACT HWDGE ring so a store waiting on DVE
                # can't head-of-line-block the next supergroup's loads (SP ring)
                nc.scalar.dma_start(res_d[:, blk0:blk0 + sz, 0:32], res[:])
                nc.scalar.dma_start(res_d[:, blk0:blk0 + sz, 32:48],
                                    rh[:, :, HS])

            total = NSG * niter
            states = {}
            for s in range(total + LAG):
                if s - LAG >= 0 and (s - LAG) in states:
                    elem_phase(states.pop(s - LAG))
                if s < total:
                    states[s] = gate_phase(s % NSG)

    nc.compile()
    return nc


def _get_nc():
    if "nc" not in _NC_CACHE:
        _NC_CACHE["nc"] = build_nc()
    return _NC_CACHE["nc"]


def gather_out(results):
    n = np.empty((B, 16), np.float32)
    h = np.empty((B, 16), np.float32)
    c = np.empty((B, 16), np.float32)
    for m in range(NCORES):
        res = np.asarray(results[m]["res"]).astype(np.float32)
        flat = res.transpose(1, 0, 2).reshape(3 * TP, 48)
        for rho in range(3):
            first = m * R + ((rho - m * R) % 3)
            T = len(range(first, (m + 1) * R, 3))
            seg = flat[rho * TP: rho * TP + T]
            n[first:(m + 1) * R:3] = seg[:, 0:16]
            c[first:(m + 1) * R:3] = seg[:, 16:32]
            h[first:(m + 1) * R:3] = seg[:, 32:48]
    return n, h, c


def make_in_maps(cores):
    keys = ["xta", "xtb", "wa", "wb", "cpt", "cnt"]
    if W_FP8 and W_HILO:
        keys.append("wal")
    return [{k: c[k] for k in keys} for c in cores]


def kernel(**inputs):
    from concourse.bass_utils import run_bass_kernel_spmd

    cores = host_prep(inputs)
    nc = _get_nc()
    out = run_bass_kernel_spmd(nc, make_in_maps(cores),
                               core_ids=list(range(NCORES)))
    return gather_out(out.results)


# revision 16
# speedup vs baseline: 6.3863x; 6.3863x over previous
"""Trainium2 Bass kernel for nn_CausalLSTMNodeCell (B=1048576, D=32, H=16, C=3).

Strategy: pure data parallel over the batch across 8 cores, with output rows
organized by residue b mod 3 so the TF-row-major child_r reshape becomes three
extra contiguous-row matmuls. Per 128-row block (K=98, block-diagonal rhs):
  psum[:, 0:32]   = xtB.T @ WB[rho]   (r1, r2 gates)
  psum[:, 32:144] = xtA.T @ WA[rho]   (r0, a, ifo, n1, n2)
All gates (incl. tanh'd "a", via tanh z = 2*sigmoid(2z)-1 with host-prescaled
weights) come out of one Sigmoid per psum group on ScalarE. Each block's psum
slice is 256 f32 wide so every matmul output stays inside one 2KB PSUM bank.

v3: fp8 E3M4 for BOTH matmul operands (uniform dtype; mixed fp8xbf16 is a
slow/incorrect path on this stack). Weights are pre-scaled x32 so they sit in
e3m4's normal range (sigmoid applies scale=1/32 for free); the precision-
critical A/I/F/O columns get a second fp8 "lo" residual matmul accumulating
into the same psum (error ~0.2%, emulated rel err n .0095 / h .012 / c .007).
Elementwise is bf16 with a reordered gate layout [r1 r2 | r0 a i f o n1 n2]:
the three r-gate muls fuse into ONE 48-col GPSIMD op, [n1 n2]*[r h] into one
32-col DVE op. Supergroups of 32 blocks are software-pipelined with LAG=1;
loads ride the SP HWDGE ring, stores the ACT ring.
"""

import numpy as np
import ml_dtypes

B, D, H, C = 1048576, 32, 16, 3
NCORES = 8
R = B // NCORES            # 131072 rows per core
TPB = 342                  # blocks per residue section (padded, 43776>=43691)
TP = TPB * 128             # rows per section
NBLK = 3 * TPB             # blocks per core
SG = 32                    # max blocks per supergroup
GRP = 8                    # blocks per psum group
SG_TABLE = []              # (blk0, size) pairs
for _rho in range(3):
    _off = _rho * TPB
    for _i in range(TPB // SG):
        SG_TABLE.append((_off + _i * SG, SG))
    if TPB % SG:
        SG_TABLE.append((_off + (TPB // SG) * SG, TPB % SG))
NSG = len(SG_TABLE)
KDIM = 98
bf16 = ml_dtypes.bfloat16
f8e4 = ml_dtypes.float8_e4m3

XB_FP8 = True              # ship xtB (r1/r2 slab) + WB as fp8 e4m3;
                           # xtA/WA stay bf16 (c/h precision path). Each
                           # matmul is dtype-uniform (mixed and e3m4 paths
                           # are slow/broken on this stack).
W_SCALE = 32.0             # weights pre-scaled x32, sigmoid scale=1/32
EL16 = True                # bf16 elementwise tiles (False: f32 like v1)
XT_BUFS = 3                # buffer depth for xtA/xtB load tiles
AXD_BUFS = 4               # buffer depth for c_prev/child load tiles
LAG = 1                    # software-pipeline depth (supergroups)

_NC_CACHE = {}


def _build_w(inputs):
    W = np.zeros((49, 144), np.float32)

    def put(cols, wx, wh, bx, bh):
        W[0:32, cols] = inputs[wx]
        W[32:48, cols] = inputs[wh]
        W[48, cols] = inputs[bx] + inputs[bh]

    put(slice(0, 48), "W_ifo_x", "W_ifo_h", "b_ifo_x", "b_ifo_h")
    put(slice(48, 64), "W_n1_x", "W_n1_h", "b_n1_x", "b_n1_h")
    put(slice(64, 80), "W_n2_x", "W_n2_h", "b_n2_x", "b_n2_h")
    put(slice(80, 96), "W_a_x", "W_a_h", "b_a_x", "b_a_h")
    put(slice(96, 144), "W_r_x", "W_r_h", "b_r_x", "b_r_h")
    return W


def host_prep(inputs):
    x = np.asarray(inputs["inputs"], np.float32)
    hp = np.asarray(inputs["h_prev"], np.float32)
    cp = np.asarray(inputs["c_prev"], np.float32)
    ch = np.asarray(inputs["child_n"], np.float32)
    W = _build_w(inputs)
    Wr = W[:, 96:144]
    xh = np.zeros((B + 1, 49), np.float32)
    xh[:B, 0:32] = x
    xh[:B, 32:48] = hp
    xh[:B, 48] = 1.0
    xh16 = xh.astype(bf16)
    xbdt = f8e4 if XB_FP8 else bf16
    xh8 = xh.astype(xbdt)

    cndt = bf16
    cp16 = cp.astype(bf16)

    cores = []
    for m in range(NCORES):
        xtA = np.zeros((KDIM, 3 * TP), bf16)
        xtB = np.zeros((KDIM, 3 * TP), xbdt)
        cpp = np.zeros((3 * TP, 16), bf16)
        # cn packed in gate order [r1 r2 r0] -> child columns [ch1 ch2 ch0]
        chp = np.zeros((3 * TP, 48), cndt)
        WA = np.zeros((3, KDIM, 112), np.float32)
        WB = np.zeros((3, KDIM, 32), np.float32)
        for rho in range(3):
            first = m * R + ((rho - m * R) % 3)
            T = len(range(first, (m + 1) * R, 3))
            sl = slice(rho * TP, rho * TP + TP)
            bidx = first + 3 * np.arange(TP)
            bidx = np.minimum(bidx, B)
            bidx[T:] = B
            xtA[0:49, sl] = xh16[bidx].T
            cpp[sl.start:sl.start + T] = cp16[first:(m + 1) * R:3]
            chp[sl.start:sl.start + T, 0:16] = \
                ch[1, first:(m + 1) * R:3].astype(cndt)
            chp[sl.start:sl.start + T, 16:32] = \
                ch[2, first:(m + 1) * R:3].astype(cndt)
            chp[sl.start:sl.start + T, 32:48] = \
                ch[0, first:(m + 1) * R:3].astype(cndt)
            q = [(k * 16 * B + 16 * first) // 48 for k in range(3)]
            c = [16 * ((k + rho) % 3) for k in range(3)]
            for k, dst, srcx, rows in ((0, xtA, xh16, slice(49, 98)),
                                       (1, xtB, xh8, slice(0, 49)),
                                       (2, xtB, xh8, slice(49, 98))):
                qi = np.minimum(q[k] + np.arange(TP), B)
                dst[rows, sl] = srcx[qi].T
            # gate column order (psum): [R1 R2 | R0 A I F O N1 N2]
            WA[rho, 49:98, 0:16] = Wr[:, c[0]:c[0] + 16]    # r0 -> psum 32:48
            # a-preact scaled by 2: tanh(z) = 2*sigmoid(2z) - 1
            WA[rho, 0:49, 16:32] = 2.0 * W[:, 80:96]        # a   -> 48:64
            WA[rho, 0:49, 32:80] = W[:, 0:48]               # ifo -> 64:112
            WA[rho, 0:49, 80:96] = W[:, 48:64]              # n1  -> 112:128
            WA[rho, 0:49, 96:112] = W[:, 64:80]             # n2  -> 128:144
            WB[rho, 0:49, 0:16] = Wr[:, c[1]:c[1] + 16]     # r1 -> 0:16
            WB[rho, 49:98, 16:32] = Wr[:, c[2]:c[2] + 16]   # r2 -> 16:32
        cpp = np.ascontiguousarray(
            cpp.reshape(NBLK, 128, 16).transpose(1, 0, 2))
        chp = np.ascontiguousarray(
            chp.reshape(NBLK, 128, 48).transpose(1, 0, 2))
        core = dict(xta=xtA, xtb=xtB, cpt=cpp, cnt=chp)
        core["wa"] = (W_SCALE * WA).astype(bf16)
        if XB_FP8:
            core["wb"] = np.clip(W_SCALE * WB, -240, 240).astype(f8e4)
        else:
            core["wb"] = (W_SCALE * WB).astype(bf16)
        cores.append(core)
    return cores


def build_nc(niter=1, sg_bufs=3):
    import concourse.tile as tile
    from concourse import bacc, mybir

    f32 = mybir.dt.float32
    b16 = mybir.dt.bfloat16
    eldt = b16 if EL16 else f32
    xbdt = mybir.dt.float8e4 if XB_FP8 else b16
    cndt = b16
    AF = mybir.ActivationFunctionType
    ALU = mybir.AluOpType

    nc = bacc.Bacc(None, target_bir_lowering=False)
    xta_d = nc.dram_tensor("xta", [KDIM, 3 * TP], b16, kind="ExternalInput")
    xtb_d = nc.dram_tensor("xtb", [KDIM, 3 * TP], xbdt, kind="ExternalInput")
    wa_d = nc.dram_tensor("wa", [3, KDIM, 112], b16, kind="ExternalInput")
    wb_d = nc.dram_tensor("wb", [3, KDIM, 32], xbdt, kind="ExternalInput")
    cpt_d = nc.dram_tensor("cpt", [128, NBLK, 16], b16, kind="ExternalInput")
    cnt_d = nc.dram_tensor("cnt", [128, NBLK, 48], cndt, kind="ExternalInput")
    # dram res columns: [n | c | h]
    res_d = nc.dram_tensor("res", [128, NBLK, 48], b16, kind="ExternalOutput")

    # gate columns (psum/GATES): [R1 R2 | R0 A I F O N1 N2]
    RRR = slice(0, 48)
    A = slice(48, 64)
    I, F, O = slice(64, 80), slice(80, 96), slice(96, 112)
    N12 = slice(112, 144)
    # tmp columns
    T0, T1, TC = slice(0, 16), slice(16, 32), slice(32, 48)
    P = slice(48, 96)
    P1, P2, P3 = slice(48, 64), slice(64, 80), slice(80, 96)
    # rh columns
    RS, HS = slice(0, 16), slice(16, 32)
    # res columns
    RN, RC = slice(0, 16), slice(16, 32)

    with tile.TileContext(nc) as tc:
        with (
            tc.tile_pool(name="wp", bufs=1) as wp,
            tc.tile_pool(name="xtab", bufs=XT_BUFS) as xtabp,
            tc.tile_pool(name="axd", bufs=AXD_BUFS) as axdp,
            tc.tile_pool(name="gates", bufs=sg_bufs) as gatesp,
            tc.tile_pool(name="tmp", bufs=sg_bufs) as tmpp,
            tc.tile_pool(name="res", bufs=sg_bufs) as resp,
            tc.tile_pool(name="ps", bufs=2, space="PSUM") as psp,
        ):
            wa_t = wp.tile([KDIM, 3, 112], b16, tag="wa")
            wb_t = wp.tile([KDIM, 3, 32], xbdt, tag="wb")
            for rho in range(3):
                nc.sync.dma_start(wa_t[:, rho, :], wa_d[rho])
                nc.sync.dma_start(wb_t[:, rho, :], wb_d[rho])

            V = nc.vector
            G = nc.gpsimd

            def gate_phase(s):
                blk0, sz = SG_TABLE[s]
                rho = blk0 // TPB
                col0 = blk0 * 128
                xta_t = xtabp.tile([KDIM, sz * 128], b16, tag="xta")
                nc.sync.dma_start(xta_t[:], xta_d[:, col0:col0 + sz * 128])
                xtb_t = xtabp.tile([KDIM, sz * 128], xbdt, tag="xtb")
                nc.sync.dma_start(xtb_t[:], xtb_d[:, col0:col0 + sz * 128])
                cpt = axdp.tile([128, sz, 16], b16, tag="cpt")
                nc.sync.dma_start(cpt[:], cpt_d[:, blk0:blk0 + sz, :])
                cnt = axdp.tile([128, sz, 48], cndt, tag="cnt")
                nc.sync.dma_start(cnt[:], cnt_d[:, blk0:blk0 + sz, :])

                gates = gatesp.tile([128, sz, 144], eldt, tag="gates")
                tmp = tmpp.tile([128, sz, 96], eldt, tag="tmp")
                rh = tmpp.tile([128, sz, 32], b16, tag="rh")
                nm = tmpp.tile([128, sz, 32], eldt, tag="nm")
                res = resp.tile([128, sz, 32], b16, tag="res")
                for g in range(-(-sz // GRP)):
                    gsz = min(GRP, sz - g * GRP)
                    # 256 f32 per block: each matmul output slice stays
                    # inside one 2KB PSUM bank (144-wide tiling would make
                    # blocks 3/6/10/13 span banks)
                    ps = psp.tile([128, gsz, 256], f32, tag="ps")
                    for bb in range(gsz):
                        k = g * GRP + bb
                        nc.tensor.matmul(
                            ps[:, bb, 0:32],
                            xtb_t[:, k * 128:(k + 1) * 128],
                            wb_t[:, rho, :])
                        nc.tensor.matmul(
                            ps[:, bb, 32:144],
                            xta_t[:, k * 128:(k + 1) * 128],
                            wa_t[:, rho, :])
                    gsl = slice(g * GRP, g * GRP + gsz)
                    nc.scalar.activation(
                        gates[:, gsl, 0:144], ps[:, :, 0:144],
                        AF.Sigmoid, scale=1.0 / W_SCALE)
                # c chain: a' = 2*sig_a - 1 ; t0 = a'*i ; t1 = f*cp ; c = t0+t1
                sl = slice(0, sz)
                V.tensor_scalar(tmp[:, sl, TC], gates[:, sl, A], 2.0,
                                -1.0, ALU.mult, ALU.add)
                V.tensor_mul(tmp[:, sl, T0], tmp[:, sl, TC], gates[:, sl, I])
                V.tensor_mul(tmp[:, sl, T1], gates[:, sl, F], cpt[:, sl, :])
                V.tensor_add(res[:, sl, RC], tmp[:, sl, T0], tmp[:, sl, T1])
                return (gates, cpt, cnt, tmp, rh, nm, res, blk0, sz)

            def elem_phase(state):
                gates, cpt, cnt, tmp, rh, nm, res, blk0, sz = state
                nc.scalar.activation(tmp[:, :, TC], res[:, :, RC], AF.Tanh)
                V.tensor_mul(rh[:, :, HS], gates[:, :, O], tmp[:, :, TC])
                G.tensor_mul(tmp[:, :, P], gates[:, :, RRR], cnt[:, :, :])
                V.tensor_add(tmp[:, :, T0], tmp[:, :, P1], tmp[:, :, P2])
                V.tensor_add(rh[:, :, RS], tmp[:, :, T0], tmp[:, :, P3])
                V.tensor_mul(nm[:, :, :], gates[:, :, N12], rh[:, :, :])
                V.tensor_add(res[:, :, RN], nm[:, :, 0:16], nm[:, :, 16:32])
                # stores ride the ACT HWDGE ring so a store waiting on DVE
                # can't head-of-line-block the next supergroup's loads (SP ring)
                nc.scalar.dma_start(res_d[:, blk0:blk0 + sz, 0:32], res[:])
                nc.scalar.dma_start(res_d[:, blk0:blk0 + sz, 32:48],
                                    rh[:, :, HS])

            total = NSG * niter
            states = {}
            for s in range(total + LAG):
                if s - LAG >= 0 and (s - LAG) in states:
                    elem_phase(states.pop(s - LAG))
                if s < total:
                    states[s] = gate_phase(s % NSG)

    nc.compile()
    return nc


def _get_nc():
    if "nc" not in _NC_CACHE:
        _NC_CACHE["nc"] = build_nc()
    return _NC_CACHE["nc"]


def gather_out(results):
    n = np.empty((B, 16), np.float32)
    h = np.empty((B, 16), np.float32)
    c = np.empty((B, 16), np.float32)
    for m in range(NCORES):
        res = np.asarray(results[m]["res"]).astype(np.float32)
        flat = res.transpose(1, 0, 2).reshape(3 * TP, 48)
        for rho in range(3):
            first = m * R + ((rho - m * R) % 3)
            T = len(range(first, (m + 1) * R, 3))
            seg = flat[rho * TP: rho * TP + T]
            n[first:(m + 1) * R:3] = seg[:, 0:16]
            c[first:(m + 1) * R:3] = seg[:, 16:32]
            h[first:(m + 1) * R:3] = seg[:, 32:48]
    return n, h, c


def make_in_maps(cores):
    keys = ["xta", "xtb", "wa", "wb", "cpt", "cnt"]
    return [{k: c[k] for k in keys} for c in cores]


def kernel(**inputs):
    from concourse.bass_utils import run_bass_kernel_spmd

    cores = host_prep(inputs)
    nc = _get_nc()
    out = run_bass_kernel_spmd(nc, make_in_maps(cores),
                               core_ids=list(range(NCORES)))
    return gather_out(out.results)
